# revision 9
# baseline (speedup 1.0000x reference)
"""Trainium2 Bass kernel for nn_MultiHeadAttention_2250562863251.

Key algebraic insight: the reference einsum 'mbhi,nbhj->mnbh' contracts i and j
independently, so scores[m,n,b,h] = (sum_i q[m,b,h,i]) * (sum_j k[n,b,h,j]) --
a rank-1 outer product of per-head row-sums. Full Q/K projections are never
needed; only queries @ (per-head-summed Wq) [E,16], computed on host (tiny).

Sharding: 8 cores = 2 (batch) x 4 (head-groups of 4 heads). SPMD program via
run_bass_kernel_spmd; host shards inputs / gathers + reduces outputs.

V2 device pipeline per core (batch bi, heads hg*4..hg*4+3), scoresT[n,m]
orientation so softmax reductions land on the PE via a ones-column:
  - v-proj (PE, bf16) first as one deep matmul burst (PE p-state ramp),
    k ascending so stage 2 consumes v_sb[0] first.
  - stage 2 runs 4 passes over (head-pair hp, m-half tp), k ascending
    8*tp..15.  Per (pass, k): two above-diagonal score stts (DVE, or GpSimd
    for the widest chunks), two diagonal-square stts vs host btri tiles
    (DVE), all writing one sc tile; ONE wide ACT exp over the whole tile
    (bias = per-partition pad mask d); per head: one stationary v load and
    up to 3 pooling matmuls into per-(head,t) psum tiles accumulated over k.
  - pass drain: denominator row 64 of each pooling psum tile is copied
    (ACT) to SBUF, DMA-gathered to a [128, 16] spread so ONE DVE reciprocal
    covers the pass, DMA re-broadcast [64, 512], and applied by DVE
    multiplies that read the pooling psum directly (no [65,512] copies).
  - out-proj (PE, bf16, K=128 per head-pair block) per m-tile as soon as
    both head-pairs' pooled tiles for that tile are done; psum -> sbuf
    copies alternate ACT/DVE; DMA out fp32.
Host assembles: out[m,b,:] = sum_hg outT.T + bo + bv @ Wo.T (softmax rows sum
to 1, so the v-bias contribution reduces to a constant), with exact numpy
recompute of (rare; absent at seed 0) degenerate rows where rowmax <= -990.
"""
import sys

for _p in ("/opt/trn_rl_repo", "/root/.axon_site/_ro/trn_rl_repo"):
    if _p not in sys.path:
        sys.path.append(_p)

import numpy as np
import ml_dtypes

import concourse.bass as bass
import concourse.mybir as mybir
import concourse.tile as tile
from concourse import bacc
from concourse.bass_utils import run_bass_kernel_spmd

# Problem shapes (hardcoded per contract)
M = 2048   # query positions
N = 2048   # key positions
B = 2
E = 1024
H = 16
DH = 64        # head dim
HL = 4         # heads per core
KL = HL * DH   # 256 local pooled dims
NEG = -1000.0
P = 128
NK = N // P    # 16 n-chunks
T = 4          # m-tiles of 512
MT = 512
NCORES = 8

f32 = mybir.dt.float32
bf16 = mybir.dt.bfloat16

_CACHE = {}

# above-chunk (pass tp, k) score-builds routed to GpSimd instead of DVE
# (empty: walrus rejects TensorScalarPtr on the Pool engine on TRN2)
GPS_ABOVE = set()


def _build_program():
    if "nc" in _CACHE:
        return _CACHE["nc"]
    nc = bacc.Bacc("TRN2", target_bir_lowering=False, debug=False,
                   num_devices=NCORES)

    vt_d = nc.declare_dram_parameter("vt", [P, 4, (E // P) * MT], bf16, isOutput=False)
    wvlt_d = nc.declare_dram_parameter("wvlt", [P, (E // P) * KL], bf16, isOutput=False)
    wolt_d = nc.declare_dram_parameter("wolt", [P, (KL // P) * E], bf16, isOutput=False)
    qsl_d = nc.declare_dram_parameter("qsl", [HL, M], f32, isOutput=False)
    betal_d = nc.declare_dram_parameter("betal", [HL, M], bf16, isOutput=False)
    cd_d = nc.declare_dram_parameter("cd", [P, NK * (HL + 1)], f32, isOutput=False)
    # per-k diagonal tiles: btrik[k][p, h*128+j] = beta_h[k*128+j] + tri(p<j)
    btrik_d = nc.declare_dram_parameter("btrik", [NK, P, HL * P], bf16,
                                        isOutput=False)
    # blocked output: [ob, t, 128, 512] -> host reassembles to [E, M]
    outp_d = nc.declare_dram_parameter("outp", [E // P, T, P, MT], f32,
                                       isOutput=True)

    with tile.TileContext(nc) as tc:
        with (
            tc.tile_pool(name="const", bufs=1) as const,
            tc.tile_pool(name="vstream", bufs=2) as vstream,
            tc.tile_pool(name="scpool", bufs=3) as scpool,
            tc.tile_pool(name="etpool", bufs=4) as etpool,
            tc.tile_pool(name="drain", bufs=2) as drain,
            tc.tile_pool(name="ptn", bufs=1) as ptn,
            tc.tile_pool(name="opool", bufs=3) as opool,
            tc.tile_pool(name="dpool", bufs=4, space="DRAM") as dpool,
            tc.tile_pool(name="ps_pool", bufs=1, space="PSUM") as ps_pool,
            tc.tile_pool(name="ps_mm", bufs=2, space="PSUM") as ps_mm,
        ):
            # ---- resident constants (bulk DMAs, few descriptors) ----
            wvlt_sb = const.tile([P, E // P, KL], bf16)
            nc.sync.dma_start(wvlt_sb[:], wvlt_d.rearrange("p (ek d) -> p ek d", ek=E // P))
            wolt_sb = const.tile([P, KL // P, E], bf16)
            nc.sync.dma_start(wolt_sb[:], wolt_d.rearrange("p (kb o) -> p kb o", kb=KL // P))
            cd_sb = const.tile([P, NK, HL + 1], f32)
            nc.sync.dma_start(cd_sb[:], cd_d.rearrange("p (k f) -> p k f", k=NK))
            btri_sb = const.tile([P, NK, HL * P], bf16)
            nc.sync.dma_start(btri_sb[:], btrik_d.rearrange("k p x -> p k x"))
            qsb = const.tile([P, HL, M], f32)
            nc.sync.dma_start(qsb[:], qsl_d[None, :, :].to_broadcast([P, HL, M]))
            bb = const.tile([P, HL, M], bf16)
            nc.sync.dma_start(bb[:], betal_d[None, :, :].to_broadcast([P, HL, M]))

            # v_sb[:, k, h*65 : h*65+64] = v for head h, chunk k; col 64 = 1.0
            v_sb = const.tile([P, NK, HL * (DH + 1)], bf16)
            nc.vector.memset(v_sb[:], 1.0)

            # ---- stage 1: v projection (ASCENDING: stage 2 consumes k=0 first)
            for q in range(4):
                vt_sb = vstream.tile([P, E // P, MT], bf16, tag="vt")
                nc.sync.dma_start(
                    vt_sb[:], vt_d[:, q].rearrange("p (ek n) -> p ek n", ek=E // P))
                for nk_r in range(4):
                    k = q * 4 + nk_r
                    vps = ps_mm.tile([P, KL], f32, tag="vps")
                    for ek in range(E // P):
                        nc.tensor.matmul(
                            vps[:],
                            vt_sb[:, ek, nk_r * P:(nk_r + 1) * P],
                            wvlt_sb[:, ek, :],
                            start=(ek == 0),
                            stop=(ek == E // P - 1),
                        )
                    nc.any.tensor_copy(
                        out=v_sb[:, k].rearrange("p (h x) -> p h x", x=DH + 1)[:, :, 0:DH],
                        in_=vps.rearrange("p (h x) -> p h x", x=DH),
                    )

            # ---- stage 2: scores / softmax / attention, 4 passes ----
            # pass (hp, tp): heads {2hp, 2hp+1}, m in [tp*1024, tp*1024+1024)
            SCW = 2048   # max sc width
            osb_flip = [0]

            def drain_pass(hp, tp, pools):
                """Denominators + divides for one pass; writes pTn2 slices."""
                # 1) ACT-copy the 4 denominator rows (partition 64) to SBUF
                drow = drain.tile([DH + 1, 4 * MT], f32, tag="drow")
                for u, (i, t) in enumerate(sorted(pools)):
                    nc.scalar.copy(drow[DH:DH + 1, u * MT:(u + 1) * MT],
                                   pools[(i, t)][DH:DH + 1, :])
                # 2) gather to DRAM, reload [128, 16] partition-spread
                rdall = dpool.tile([1, 4 * MT], f32, tag="rdall")
                nc.sync.dma_start(rdall[:], drow[DH:DH + 1, :])
                rsg = drain.tile([P, 4 * MT // P], f32, tag="rsg")
                nc.sync.dma_start(
                    rsg[:], rdall.rearrange("a (b x) -> (a b) x", x=4 * MT // P))
                rsgr = drain.tile([P, 4 * MT // P], f32, tag="rsgr")
                nc.vector.reciprocal(out=rsgr[:], in_=rsg[:])
                rdall2 = dpool.tile([4, MT], f32, tag="rdall2")
                nc.sync.dma_start(
                    rdall2.rearrange("a (b x) -> (a b) x", x=4 * MT // P), rsgr[:])
                # 3) per (head,t): broadcast recip + DVE mul reading psum
                for u, (i, t) in enumerate(sorted(pools)):
                    rsb = drain.tile([DH, MT], f32, tag=f"rsb{u % 2}")
                    nc.sync.dma_start(rsb[:], rdall2[u][None, :].to_broadcast([DH, MT]))
                    pm = ptn.tile([DH, MT], bf16, tag=f"ptn{u % 2}")
                    nc.vector.tensor_mul(
                        out=pm[:], in0=pools[(i, t)][0:DH, :], in1=rsb[:])
                    # pair-merge into pTn2[kb=hp]: head-pair block, 128 partitions
                    nc.sync.dma_start(
                        pTn2[i * DH:(i + 1) * DH, hp, t * MT:(t + 1) * MT], pm[:])

            def outproj(t):
                for ob in range(E // P):
                    ops = ps_mm.tile([P, MT], f32, tag="ops")
                    for kb in range(KL // P):
                        nc.tensor.matmul(
                            ops[:],
                            wolt_sb[:, kb, ob * P:(ob + 1) * P],
                            pTn2[:, kb, t * MT:(t + 1) * MT],
                            start=(kb == 0),
                            stop=(kb == KL // P - 1),
                        )
                    osb = opool.tile([P, MT], f32, tag="osb")
                    if osb_flip[0] % 2 == 0:
                        nc.vector.tensor_copy(out=osb[:], in_=ops[:])
                    else:
                        nc.scalar.copy(osb[:], ops[:])
                    osb_flip[0] += 1
                    nc.sync.dma_start(outp_d[ob, t], osb[:])

            pTn2 = const.tile([P, KL // P, M], bf16)

            for tp in range(2):
                for hp in range(2):
                    mlo, mhi = tp * 1024, tp * 1024 + 1024
                    pools, started = {}, set()
                    for k in range(8 * tp, NK):
                        dlo = P * k                      # diag m-start
                        A0 = max(0, min(dlo, mhi) - mlo)  # above width in pass
                        has_diag = mlo <= dlo < mhi
                        W2 = 2 * A0 + (2 * P if has_diag else 0)
                        sc = scpool.tile([P, SCW], bf16, tag="sc")
                        for i in range(2):
                            h = 2 * hp + i
                            if A0:
                                eng = (nc.gpsimd if (tp, k) in GPS_ABOVE
                                       else nc.vector)
                                eng.scalar_tensor_tensor(
                                    out=sc[:, i * A0:(i + 1) * A0],
                                    in0=qsb[:, h, mlo:mlo + A0],
                                    scalar=cd_sb[:, k, h:h + 1],
                                    in1=bb[:, h, mlo:mlo + A0],
                                    op0=mybir.AluOpType.mult,
                                    op1=mybir.AluOpType.add,
                                )
                            if has_diag:
                                nc.vector.scalar_tensor_tensor(
                                    out=sc[:, 2 * A0 + i * P:2 * A0 + (i + 1) * P],
                                    in0=qsb[:, h, dlo:dlo + P],
                                    scalar=cd_sb[:, k, h:h + 1],
                                    in1=btri_sb[:, k, h * P:(h + 1) * P],
                                    op0=mybir.AluOpType.mult,
                                    op1=mybir.AluOpType.add,
                                )
                        et = etpool.tile([P, SCW], bf16, tag="et")
                        nc.scalar.activation(
                            et[:, 0:W2], sc[:, 0:W2],
                            mybir.ActivationFunctionType.Exp,
                            bias=cd_sb[:, k, HL:HL + 1],
                        )
                        for i in range(2):
                            h = 2 * hp + i
                            stat = v_sb[:, k, h * (DH + 1):(h + 1) * (DH + 1)]
                            for t in (2 * tp, 2 * tp + 1):
                                if t * MT >= (k + 1) * P:
                                    continue  # slice entirely below diagonal
                                if (i, t) not in pools:
                                    pools[(i, t)] = ps_pool.tile(
                                        [DH + 1, MT], f32, tag=f"p{i}{t % 2}",
                                        name=f"pool_{i}_{t}")
                                pool_t = pools[(i, t)]
                                # above part within this slice
                                wA = min(A0 + mlo, t * MT + MT) - t * MT
                                diag_here = has_diag and dlo // MT == t
                                if wA > 0:
                                    nc.tensor.matmul(
                                        pool_t[:, 0:wA],
                                        stat,
                                        et[:, i * A0 + (t * MT - mlo):
                                           i * A0 + (t * MT - mlo) + wA],
                                        start=((i, t) not in started),
                                        stop=(k == NK - 1 and not diag_here),
                                    )
                                    started.add((i, t))
                                if diag_here:
                                    nc.tensor.matmul(
                                        pool_t[:, dlo - t * MT:dlo - t * MT + P],
                                        stat,
                                        et[:, 2 * A0 + i * P:2 * A0 + (i + 1) * P],
                                        start=((i, t) not in started),
                                        stop=(k == NK - 1),
                                    )
                                    started.add((i, t))
                    drain_pass(hp, tp, pools)
                    if hp == 1:
                        outproj(2 * tp)
                        outproj(2 * tp + 1)

    nc.compile()
    _CACHE["nc"] = nc
    return nc


def _host_prep(queries, keys, values, Wq, bq, Wk, bk, Wv, bv, Wo, bo, in_mask):
    """Host-side prep. Returns (in_maps, fixup, extras)."""
    qs = np.einsum("mbe,he->mbh", queries, Wq.reshape(H, DH, E).sum(1),
                   dtype=np.float32) + bq.reshape(H, DH).sum(1)
    ks = np.einsum("nbe,he->nbh", keys, Wk.reshape(H, DH, E).sum(1),
                   dtype=np.float32) + bk.reshape(H, DH).sum(1)

    mask3 = in_mask[:, :, None]
    cp = np.where(mask3, 0.0, ks).astype(np.float32)          # [n, b, H]
    d = np.where(in_mask, NEG, 0.0).astype(np.float32)        # [n, b]

    cmax = np.where(mask3, -np.inf, ks)
    cmax = np.maximum.accumulate(cmax[::-1], axis=0)[::-1]    # suffix max, n>=m
    cmin = np.where(mask3, np.inf, ks)
    cmin = np.minimum.accumulate(cmin[::-1], axis=0)[::-1]
    nonempty = np.maximum.accumulate((~in_mask)[::-1], axis=0)[::-1]  # [n, b]

    with np.errstate(invalid="ignore"):
        A = np.where(qs >= 0, qs * cmax, qs * cmin)           # [m, b, H]
    A = np.where(nonempty[:, :, None], A, -np.inf)
    fixup_rows = np.any(~(A > -990.0), axis=2)                # [m, b] (nan-safe)
    beta = np.where(np.isfinite(A), -A, 1e4)
    beta = np.where(np.any(~(A > -990.0), axis=2)[:, :, None], -1e4, beta)
    beta = beta.astype(np.float32)

    in_maps = []
    def pmajor(a, p=P):
        """[X*p, Y] -> [p, X*Y]: partition-major packing for 1-run-per-
        partition DMA loads matching 'p (x y) -> p x y' device views."""
        X = a.shape[0] // p
        return np.ascontiguousarray(
            a.reshape(X, p, a.shape[1]).transpose(1, 0, 2).reshape(p, -1))

    def pack_vt(vT):
        # [E, N] -> [P, 4, (E//P)*MT]: quarter-major, then ek-major
        a = vT.reshape(E // P, P, 4, MT)          # [ek, p, q, mt]
        return np.ascontiguousarray(
            a.transpose(1, 2, 0, 3).reshape(P, 4, (E // P) * MT))

    vt_by_b = [pack_vt(values[:, bi, :].T.astype(ml_dtypes.bfloat16))
               for bi in range(B)]
    # tri[p, j] = -4000 where n (=k*128+p) < m (=k*128+j), i.e. p < j
    tri = np.where(np.arange(P)[:, None] < np.arange(P)[None, :],
                   -4000.0, 0.0).astype(np.float32)

    for c in range(NCORES):
        bi, hg = c // 4, c % 4
        lh = slice(hg * HL, (hg + 1) * HL)
        ds = slice(hg * KL, (hg + 1) * KL)
        beta_lh = beta[:, bi, lh]                              # [M, HL]
        # btrik[k][p, h*128+j] = beta_h[k*128+j] + tri[p, j]
        bt = (beta_lh.reshape(NK, P, HL).transpose(0, 2, 1)[:, None, :, :]
              + tri[:, None, :])                               # [NK, P, HL, P]
        btrik = np.ascontiguousarray(
            bt.reshape(NK, P, HL * P)).astype(ml_dtypes.bfloat16)
        in_maps.append({
            "vt": vt_by_b[bi],
            "wvlt": pmajor(Wv[ds, :].T.astype(ml_dtypes.bfloat16)),
            "wolt": pmajor(Wo[:, ds].T.astype(ml_dtypes.bfloat16)),
            "qsl": np.ascontiguousarray(qs[:, bi, lh].T),
            "betal": np.ascontiguousarray(beta_lh.T).astype(ml_dtypes.bfloat16),
            "cd": pmajor(np.ascontiguousarray(
                np.concatenate([cp[:, bi, lh], d[:, bi:bi + 1]], axis=1))),
            "btrik": btrik,
        })
    return in_maps, fixup_rows, (qs, ks)


def _fixup_row(out, m, bi, qs, ks, values, Wv, bv, Wo, bo, in_mask):
    """Exact numpy recompute of one output row (degenerate / extreme rows)."""
    pot = qs[m, bi, :][None, :] * ks[:, bi, :]                # [n, H]
    pot = np.where(in_mask[:, bi][:, None], NEG, pot)
    causal = np.arange(N) < m                                 # mask n < m
    pot = np.where(causal[:, None], NEG, pot)
    pot = pot - pot.max(axis=0, keepdims=True)
    w = np.exp(pot)
    w = w / w.sum(axis=0, keepdims=True)                      # [n, H]
    v = (values[:, bi, :] @ Wv.T + bv).reshape(N, H, DH)
    pooled = np.einsum("nh,nhd->hd", w, v).reshape(E)
    out[m, bi, :] = pooled @ Wo.T + bo


def kernel(queries, keys, values, Wq, bq, Wk, bk, Wv, bv, Wo, bo, in_mask,
           _trace=False):
    args = (queries, keys, values, Wq, bq, Wk, bk, Wv, bv, Wo, bo)
    args = tuple(np.asarray(a, np.float32) for a in args)
    in_mask = np.asarray(in_mask, bool)
    (queries, keys, values, Wq, bq, Wk, bk, Wv, bv, Wo, bo) = args

    nc = _build_program()
    in_maps, fixup_rows, (qs, ks) = _host_prep(
        queries, keys, values, Wq, bq, Wk, bk, Wv, bv, Wo, bo, in_mask)

    res = run_bass_kernel_spmd(nc, in_maps, list(range(NCORES)), trace=_trace)
    results = res.results

    out = np.zeros((M, B, E), np.float32)
    for c in range(NCORES):
        bi = c // 4
        blk = np.asarray(results[c]["outp"], np.float32)   # [8, 4, 128, 512]
        outT = blk.transpose(0, 2, 1, 3).reshape(E, M)
        out[:, bi, :] += outT.T
    out += (bo + bv @ Wo.T)[None, None, :]

    for m, bi in zip(*np.nonzero(fixup_rows)):
        _fixup_row(out, m, bi, qs, ks, values, Wv, bv, Wo, bo, in_mask)

    if _trace:
        return out, res
    return out


# revision 16
# speedup vs baseline: 1.0936x; 1.0936x over previous
"""Trainium2 Bass kernel for nn_MultiHeadAttention_2250562863251.

Key algebraic insight: the reference einsum 'mbhi,nbhj->mnbh' contracts i and j
independently, so scores[m,n,b,h] = (sum_i q[m,b,h,i]) * (sum_j k[n,b,h,j]) --
a rank-1 outer product of per-head row-sums. Full Q/K projections are never
needed; only queries @ (per-head-summed Wq) [E,16], computed on host (tiny).

Sharding: 8 cores = 2 (batch) x 4 (head-groups of 4 heads). SPMD program via
run_bass_kernel_spmd; host shards inputs / gathers + reduces outputs.

V2 device pipeline per core (batch bi, heads hg*4..hg*4+3), scoresT[n,m]
orientation so softmax reductions land on the PE via a ones-column:
  - v-proj (PE, bf16) first as one deep matmul burst (PE p-state ramp),
    k ascending so stage 2 consumes v_sb[0] first.
  - stage 2 runs 4 passes over (head-pair hp, m-half tp), k ascending
    8*tp..15.  Per (pass, k): two above-diagonal score stts (DVE, or GpSimd
    for the widest chunks), two diagonal-square stts vs host btri tiles
    (DVE), all writing one sc tile; ONE wide ACT exp over the whole tile
    (bias = per-partition pad mask d); per head: one stationary v load and
    up to 3 pooling matmuls into per-(head,t) psum tiles accumulated over k.
  - pass drain: denominator row 64 of each pooling psum tile is copied
    (ACT) to SBUF, DMA-gathered to a [128, 16] spread so ONE DVE reciprocal
    covers the pass, DMA re-broadcast [64, 512], and applied by DVE
    multiplies that read the pooling psum directly (no [65,512] copies).
  - out-proj (PE, bf16, K=128 per head-pair block) per m-tile as soon as
    both head-pairs' pooled tiles for that tile are done; psum -> sbuf
    copies alternate ACT/DVE; DMA out fp32.
Host assembles: out[m,b,:] = sum_hg outT.T + bo + bv @ Wo.T (softmax rows sum
to 1, so the v-bias contribution reduces to a constant), with exact numpy
recompute of (rare; absent at seed 0) degenerate rows where rowmax <= -990.
"""
import sys

for _p in ("/opt/trn_rl_repo", "/root/.axon_site/_ro/trn_rl_repo"):
    if _p not in sys.path:
        sys.path.append(_p)

import numpy as np
import ml_dtypes

import concourse.bass as bass
import concourse.mybir as mybir
import concourse.tile as tile
from concourse import bacc
from concourse.bass_utils import run_bass_kernel_spmd

# Problem shapes (hardcoded per contract)
M = 2048   # query positions
N = 2048   # key positions
B = 2
E = 1024
H = 16
DH = 64        # head dim
HL = 4         # heads per core
KL = HL * DH   # 256 local pooled dims
NEG = -1000.0
P = 128
NK = N // P    # 16 n-chunks
T = 4          # m-tiles of 512
MT = 512
NCORES = 8

f32 = mybir.dt.float32
bf16 = mybir.dt.bfloat16

_CACHE = {}

# above-chunk (pass tp, k) score-builds routed to GpSimd instead of DVE
# (empty: walrus rejects TensorScalarPtr on the Pool engine on TRN2)
GPS_ABOVE = set()


def _build_program():
    if "nc" in _CACHE:
        return _CACHE["nc"]
    nc = bacc.Bacc("TRN2", target_bir_lowering=False, debug=False,
                   num_devices=NCORES)

    vt_d = nc.declare_dram_parameter("vt", [P, 4, (E // P) * MT], bf16, isOutput=False)
    wvlt_d = nc.declare_dram_parameter("wvlt", [P, (E // P) * KL], bf16, isOutput=False)
    wolt_d = nc.declare_dram_parameter("wolt", [P, (KL // P) * E], bf16, isOutput=False)
    qsl_d = nc.declare_dram_parameter("qsl", [HL, M], f32, isOutput=False)
    betal_d = nc.declare_dram_parameter("betal", [HL, M], bf16, isOutput=False)
    cd_d = nc.declare_dram_parameter("cd", [P, NK * (HL + 1)], f32, isOutput=False)
    # per-k diagonal tiles: btrik[k][p, h*128+j] = beta_h[k*128+j] + tri(p<j)
    btrik_d = nc.declare_dram_parameter("btrik", [NK, P, HL * P], bf16,
                                        isOutput=False)
    # blocked output: [ob, t, 128, 512] -> host reassembles to [E, M]
    outp_d = nc.declare_dram_parameter("outp", [E // P, T, P, MT], f32,
                                       isOutput=True)

    with tile.TileContext(nc) as tc:
        with (
            tc.tile_pool(name="const", bufs=1) as const,
            tc.tile_pool(name="vstream", bufs=2) as vstream,
            tc.tile_pool(name="scpool", bufs=3) as scpool,
            tc.tile_pool(name="etpool", bufs=6) as etpool,
            tc.tile_pool(name="drain", bufs=2) as drain,
            tc.tile_pool(name="rspool", bufs=2) as rspool,
            tc.tile_pool(name="ptn", bufs=2) as ptn,
            tc.tile_pool(name="opool", bufs=3) as opool,
            tc.tile_pool(name="dpool", bufs=4, space="DRAM") as dpool,
            tc.tile_pool(name="ps_pool", bufs=1, space="PSUM") as ps_pool,
            tc.tile_pool(name="ps_mm", bufs=2, space="PSUM") as ps_mm,
        ):
            # ---- resident constants (bulk DMAs, few descriptors) ----
            wvlt_sb = const.tile([P, E // P, KL], bf16)
            nc.sync.dma_start(wvlt_sb[:], wvlt_d.rearrange("p (ek d) -> p ek d", ek=E // P))
            wolt_sb = const.tile([P, KL // P, E], bf16)
            nc.sync.dma_start(wolt_sb[:], wolt_d.rearrange("p (kb o) -> p kb o", kb=KL // P))
            cd_sb = const.tile([P, NK, HL + 1], f32)
            nc.sync.dma_start(cd_sb[:], cd_d.rearrange("p (k f) -> p k f", k=NK))
            btri_sb = const.tile([P, NK, HL * P], bf16)
            nc.sync.dma_start(btri_sb[:], btrik_d.rearrange("k p x -> p k x"))
            qsb = const.tile([P, HL, M], f32)
            nc.sync.dma_start(qsb[:], qsl_d[None, :, :].to_broadcast([P, HL, M]))
            bb = const.tile([P, HL, M], bf16)
            nc.sync.dma_start(bb[:], betal_d[None, :, :].to_broadcast([P, HL, M]))

            # v_sb[:, k, h*65 : h*65+64] = v for head h, chunk k; col 64 = 1.0
            v_sb = const.tile([P, NK, HL * (DH + 1)], bf16)
            nc.gpsimd.memset(v_sb[:], 1.0)

            # ---- stage 1: v projection (ASCENDING: stage 2 consumes k=0 first)
            for q in range(4):
                vt_sb = vstream.tile([P, E // P, MT], bf16, tag="vt")
                nc.sync.dma_start(
                    vt_sb[:], vt_d[:, q].rearrange("p (ek n) -> p ek n", ek=E // P))
                for nk_r in range(4):
                    k = q * 4 + nk_r
                    vps = ps_mm.tile([P, KL], f32, tag="vps")
                    for ek in range(E // P):
                        nc.tensor.matmul(
                            vps[:],
                            vt_sb[:, ek, nk_r * P:(nk_r + 1) * P],
                            wvlt_sb[:, ek, :],
                            start=(ek == 0),
                            stop=(ek == E // P - 1),
                        )
                    nc.scalar.copy(
                        v_sb[:, k].rearrange("p (h x) -> p h x", x=DH + 1)[:, :, 0:DH],
                        vps.rearrange("p (h x) -> p h x", x=DH),
                    )

            # ---- stage 2: scores / softmax / attention, 4 passes ----
            # pass (hp, tp): heads {2hp, 2hp+1}, m in [tp*1024, tp*1024+1024)
            SCW = 2048   # max sc width
            osb_flip = [0]

            def drain_pass(hp, tp, pools):
                """Denominators + divides for one pass; writes pTn2 slices."""
                # 1) ACT-copy pooled psum (incl. denominator row 64) to SBUF,
                #    freeing the psum tile for the next pass immediately.
                pool_sbs = {}
                for u, (i, t) in enumerate(sorted(pools)):
                    psb = rspool.tile([DH + 1, MT], f32, tag=f"psb{u % 2}",
                                      name=f"psb_{i}_{t}")
                    nc.scalar.copy(psb[:], pools[(i, t)][:])
                    pool_sbs[(i, t)] = psb
                # 2) gather denom rows to DRAM, reload [128, 16] spread
                rdall = dpool.tile([4, MT], f32, tag="rdall")
                for u, (i, t) in enumerate(sorted(pools)):
                    nc.sync.dma_start(rdall[u:u + 1, :],
                                      pool_sbs[(i, t)][DH:DH + 1, :])
                rsg = drain.tile([P, 4 * MT // P], f32, tag="rsg")
                nc.sync.dma_start(
                    rsg[:], rdall.rearrange("a (b x) -> (a b) x", x=4 * MT // P))
                rsgr = drain.tile([P, 4 * MT // P], f32, tag="rsgr")
                nc.vector.reciprocal(out=rsgr[:], in_=rsg[:])
                rdall2 = dpool.tile([4, MT], f32, tag="rdall2")
                nc.sync.dma_start(
                    rdall2.rearrange("a (b x) -> (a b) x", x=4 * MT // P), rsgr[:])
                # 3) per (head,t): broadcast recip + GpSimd mul (SBUF only)
                for u, (i, t) in enumerate(sorted(pools)):
                    rsb = drain.tile([DH, MT], f32, tag=f"rsb{u % 2}")
                    nc.sync.dma_start(rsb[:], rdall2[u][None, :].to_broadcast([DH, MT]))
                    pm = ptn.tile([DH, MT], bf16, tag=f"ptn{u % 2}")
                    nc.gpsimd.tensor_mul(pm[:], pool_sbs[(i, t)][0:DH, :], rsb[:])
                    # pair-merge into pTn2[kb=hp]: head-pair block, 128 partitions
                    nc.sync.dma_start(
                        pTn2[i * DH:(i + 1) * DH, hp, t * MT:(t + 1) * MT], pm[:])

            def outproj(t):
                for ob in range(E // P):
                    ops = ps_mm.tile([P, MT], f32, tag="ops")
                    for kb in range(KL // P):
                        nc.tensor.matmul(
                            ops[:],
                            wolt_sb[:, kb, ob * P:(ob + 1) * P],
                            pTn2[:, kb, t * MT:(t + 1) * MT],
                            start=(kb == 0),
                            stop=(kb == KL // P - 1),
                        )
                    osb = opool.tile([P, MT], f32, tag="osb")
                    if osb_flip[0] % 2 == 0:
                        nc.vector.tensor_copy(out=osb[:], in_=ops[:])
                    else:
                        nc.scalar.copy(osb[:], ops[:])
                    osb_flip[0] += 1
                    nc.sync.dma_start(outp_d[ob, t], osb[:])

            pTn2 = const.tile([P, KL // P, M], bf16)

            for tp in range(2):
                for hp in range(2):
                    mlo, mhi = tp * 1024, tp * 1024 + 1024
                    pools, started = {}, set()
                    for k in range(8 * tp, NK):
                        dlo = P * k                      # diag m-start
                        A0 = max(0, min(dlo, mhi) - mlo)  # above width in pass
                        has_diag = mlo <= dlo < mhi
                        D = P if has_diag else 0
                        AD = A0 + D                      # per-head sc width
                        W2 = 2 * AD
                        sc = scpool.tile([P, SCW], bf16, tag="sc")
                        for i in range(2):
                            h = 2 * hp + i
                            if A0:
                                nc.vector.scalar_tensor_tensor(
                                    out=sc[:, i * AD:i * AD + A0],
                                    in0=qsb[:, h, mlo:mlo + A0],
                                    scalar=cd_sb[:, k, h:h + 1],
                                    in1=bb[:, h, mlo:mlo + A0],
                                    op0=mybir.AluOpType.mult,
                                    op1=mybir.AluOpType.add,
                                )
                            if has_diag:
                                nc.vector.scalar_tensor_tensor(
                                    out=sc[:, i * AD + A0:(i + 1) * AD],
                                    in0=qsb[:, h, dlo:dlo + P],
                                    scalar=cd_sb[:, k, h:h + 1],
                                    in1=btri_sb[:, k, h * P:(h + 1) * P],
                                    op0=mybir.AluOpType.mult,
                                    op1=mybir.AluOpType.add,
                                )
                        et = etpool.tile([P, SCW], bf16, tag="et")
                        nc.scalar.activation(
                            et[:, 0:W2], sc[:, 0:W2],
                            mybir.ActivationFunctionType.Exp,
                            bias=cd_sb[:, k, HL:HL + 1],
                        )
                        # et col for head i, position m: i*AD + (m - mlo)
                        # (diag cols sit right after that head's above cols)
                        for i in range(2):
                            h = 2 * hp + i
                            stat = v_sb[:, k, h * (DH + 1):(h + 1) * (DH + 1)]
                            first_mm = True
                            for t in (2 * tp, 2 * tp + 1):
                                if t * MT >= (k + 1) * P:
                                    continue  # slice entirely below diagonal
                                if (i, t) not in pools:
                                    pools[(i, t)] = ps_pool.tile(
                                        [DH + 1, MT], f32, tag=f"p{i}{t % 2}",
                                        name=f"pool_{i}_{t}")
                                pool_t = pools[(i, t)]
                                # covered m range within this slice (above
                                # plus contiguous diag if it lands here)
                                hi = min(mlo + A0 + D, t * MT + MT)
                                w = hi - t * MT
                                if w <= 0:
                                    continue
                                mm = nc.tensor.matmul(
                                    pool_t[:, 0:w],
                                    stat,
                                    et[:, i * AD + (t * MT - mlo):
                                       i * AD + (t * MT - mlo) + w],
                                    start=((i, t) not in started),
                                    stop=(k == NK - 1),
                                )
                                if not first_mm:
                                    mm.ins.ldweights = False
                                first_mm = False
                                started.add((i, t))
                    drain_pass(hp, tp, pools)
                    if hp == 1:
                        outproj(2 * tp)
                        outproj(2 * tp + 1)

    nc.compile()
    _CACHE["nc"] = nc
    return nc


def _host_prep(queries, keys, values, Wq, bq, Wk, bk, Wv, bv, Wo, bo, in_mask):
    """Host-side prep. Returns (in_maps, fixup, extras)."""
    qs = np.einsum("mbe,he->mbh", queries, Wq.reshape(H, DH, E).sum(1),
                   dtype=np.float32) + bq.reshape(H, DH).sum(1)
    ks = np.einsum("nbe,he->nbh", keys, Wk.reshape(H, DH, E).sum(1),
                   dtype=np.float32) + bk.reshape(H, DH).sum(1)

    mask3 = in_mask[:, :, None]
    cp = np.where(mask3, 0.0, ks).astype(np.float32)          # [n, b, H]
    d = np.where(in_mask, NEG, 0.0).astype(np.float32)        # [n, b]

    cmax = np.where(mask3, -np.inf, ks)
    cmax = np.maximum.accumulate(cmax[::-1], axis=0)[::-1]    # suffix max, n>=m
    cmin = np.where(mask3, np.inf, ks)
    cmin = np.minimum.accumulate(cmin[::-1], axis=0)[::-1]
    nonempty = np.maximum.accumulate((~in_mask)[::-1], axis=0)[::-1]  # [n, b]

    with np.errstate(invalid="ignore"):
        A = np.where(qs >= 0, qs * cmax, qs * cmin)           # [m, b, H]
    A = np.where(nonempty[:, :, None], A, -np.inf)
    fixup_rows = np.any(~(A > -990.0), axis=2)                # [m, b] (nan-safe)
    beta = np.where(np.isfinite(A), -A, 1e4)
    beta = np.where(np.any(~(A > -990.0), axis=2)[:, :, None], -1e4, beta)
    beta = beta.astype(np.float32)

    in_maps = []
    def pmajor(a, p=P):
        """[X*p, Y] -> [p, X*Y]: partition-major packing for 1-run-per-
        partition DMA loads matching 'p (x y) -> p x y' device views."""
        X = a.shape[0] // p
        return np.ascontiguousarray(
            a.reshape(X, p, a.shape[1]).transpose(1, 0, 2).reshape(p, -1))

    def pack_vt(vT):
        # [E, N] -> [P, 4, (E//P)*MT]: quarter-major, then ek-major
        a = vT.reshape(E // P, P, 4, MT)          # [ek, p, q, mt]
        return np.ascontiguousarray(
            a.transpose(1, 2, 0, 3).reshape(P, 4, (E // P) * MT))

    vt_by_b = [pack_vt(values[:, bi, :].T.astype(ml_dtypes.bfloat16))
               for bi in range(B)]
    # tri[p, j] = -4000 where n (=k*128+p) < m (=k*128+j), i.e. p < j
    tri = np.where(np.arange(P)[:, None] < np.arange(P)[None, :],
                   -4000.0, 0.0).astype(np.float32)

    for c in range(NCORES):
        bi, hg = c // 4, c % 4
        lh = slice(hg * HL, (hg + 1) * HL)
        ds = slice(hg * KL, (hg + 1) * KL)
        beta_lh = beta[:, bi, lh]                              # [M, HL]
        # btrik[k][p, h*128+j] = beta_h[k*128+j] + tri[p, j]
        bt = (beta_lh.reshape(NK, P, HL).transpose(0, 2, 1)[:, None, :, :]
              + tri[:, None, :])                               # [NK, P, HL, P]
        btrik = np.ascontiguousarray(
            bt.reshape(NK, P, HL * P)).astype(ml_dtypes.bfloat16)
        in_maps.append({
            "vt": vt_by_b[bi],
            "wvlt": pmajor(Wv[ds, :].T.astype(ml_dtypes.bfloat16)),
            "wolt": pmajor(Wo[:, ds].T.astype(ml_dtypes.bfloat16)),
            "qsl": np.ascontiguousarray(qs[:, bi, lh].T),
            "betal": np.ascontiguousarray(beta_lh.T).astype(ml_dtypes.bfloat16),
            "cd": pmajor(np.ascontiguousarray(
                np.concatenate([cp[:, bi, lh], d[:, bi:bi + 1]], axis=1))),
            "btrik": btrik,
        })
    return in_maps, fixup_rows, (qs, ks)


def _fixup_row(out, m, bi, qs, ks, values, Wv, bv, Wo, bo, in_mask):
    """Exact numpy recompute of one output row (degenerate / extreme rows)."""
    pot = qs[m, bi, :][None, :] * ks[:, bi, :]                # [n, H]
    pot = np.where(in_mask[:, bi][:, None], NEG, pot)
    causal = np.arange(N) < m                                 # mask n < m
    pot = np.where(causal[:, None], NEG, pot)
    pot = pot - pot.max(axis=0, keepdims=True)
    w = np.exp(pot)
    w = w / w.sum(axis=0, keepdims=True)                      # [n, H]
    v = (values[:, bi, :] @ Wv.T + bv).reshape(N, H, DH)
    pooled = np.einsum("nh,nhd->hd", w, v).reshape(E)
    out[m, bi, :] = pooled @ Wo.T + bo


def kernel(queries, keys, values, Wq, bq, Wk, bk, Wv, bv, Wo, bo, in_mask,
           _trace=False):
    args = (queries, keys, values, Wq, bq, Wk, bk, Wv, bv, Wo, bo)
    args = tuple(np.asarray(a, np.float32) for a in args)
    in_mask = np.asarray(in_mask, bool)
    (queries, keys, values, Wq, bq, Wk, bk, Wv, bv, Wo, bo) = args

    nc = _build_program()
    in_maps, fixup_rows, (qs, ks) = _host_prep(
        queries, keys, values, Wq, bq, Wk, bk, Wv, bv, Wo, bo, in_mask)

    res = run_bass_kernel_spmd(nc, in_maps, list(range(NCORES)), trace=_trace)
    results = res.results

    out = np.zeros((M, B, E), np.float32)
    for c in range(NCORES):
        bi = c // 4
        blk = np.asarray(results[c]["outp"], np.float32)   # [8, 4, 128, 512]
        outT = blk.transpose(0, 2, 1, 3).reshape(E, M)
        out[:, bi, :] += outT.T
    out += (bo + bv @ Wo.T)[None, None, :]

    for m, bi in zip(*np.nonzero(fixup_rows)):
        _fixup_row(out, m, bi, qs, ks, values, Wv, bv, Wo, bo, in_mask)

    if _trace:
        return out, res
    return out
